# revision 18
# baseline (speedup 1.0000x reference)
"""BiMamba (bidirectional Mamba block) on 8 TRN2 NeuronCores.

Sharding: 4 independent (batch, direction) units x 2-way split of
d_inner (2048 -> 2x1024). Core c = (b=c//4, dir=(c//2)%2, half=c%2).
All cores run ONE SPMD program; per-core differences are folded into the
host-prepared inputs (x transposed/reversed, weights sliced so the
core's own d_inner half is channels 0..1023). Each core computes a
full-depth partial of out[b] over its half; the host sums partials,
un-reverses the reverse direction, adds directions.

Each core computes in_proj/conv only for its OWN d_inner half; the
x_dbl projection (which contracts over all of d_inner) is computed as a
partial sum and pair-wise AllReduced between the two half-cores of each
(batch, direction) unit.

Per-core pipeline:
  A: in_proj own half + z (PE fp16); causal dw-conv as 4 shifted
     scalar-tensor ops on DVE; Silu (ACT)
  B: partial x_dbl = Wx[:, own] @ xc (PE); pair AllReduce (DRAM f16);
     dt = softplus(Wdt @ x_dbl + bdt) (PE + ACT Exp/Ln);
     B/C rows replicated to 128 partitions via 0-stride DMA
  C (per d-tile pair, per n): dA = Exp(dt*A[d,n]) (ACT per-partition
     scale); dBu = (dt*u) o B_n (DVE TT fp16 2x); h = tensor_tensor_scan
     (DVE); G = h o C_n (DVE); y = sum_n G_n + diag(Dskip) @ u via
     PSUM-accumulated matmuls (PE); y2 = y_psum * silu(z) (DVE)
  D: out_proj partial (PE) -> DRAM fp32
"""
import os
import sys
import types

sys.path.insert(0, "/opt/trn_rl_repo")

import numpy as np

# ---- NTFF profile hook shim (trace path only; harmless otherwise) ----
if "antenv.axon_hooks" not in sys.modules:
    _m = types.ModuleType("antenv.axon_hooks")
    _m._HOOK = None
    _m.set_axon_ntff_profile_hook = lambda h, _m=_m: setattr(_m, "_HOOK", h)
    _m.get_axon_ntff_profile_hook = lambda _m=_m: _m._HOOK
    sys.modules["antenv.axon_hooks"] = _m

import concourse.bacc as bacc
import concourse.tile as tile
from concourse import mybir
from concourse.bass_utils import run_bass_kernel_spmd

f32 = mybir.dt.float32
f16 = mybir.dt.float16

DT_RANK = 64
N_STATE = 16
K_CONV = 4
P = 128


def build(L=1024, DM=1024, DH=1024):
    MULT = mybir.AluOpType.mult
    ADD = mybir.AluOpType.add
    ACT = mybir.ActivationFunctionType

    nc = bacc.Bacc("TRN2", num_devices=8)
    KT = DM // P                     # k-tiles over d_model (8)
    XT = DH // P                     # own-half xi tiles (8)
    ZT = DH // P                     # z / scan tiles (8)
    ET = XT + ZT                     # in_proj e-tiles (xi own + z own)
    FD = 512                         # matmul free-dim (one PSUM bank fp32)
    NF = L // FD
    NX = DT_RANK + 2 * N_STATE       # 96

    xT = nc.dram_tensor("xT", [DM, L], f16, kind="ExternalInput")
    winT = nc.dram_tensor("winT", [P, ET, KT, P], f16, kind="ExternalInput")
    wcv = nc.dram_tensor("wcv", [P, XT, K_CONV], f32, kind="ExternalInput")
    bconv = nc.dram_tensor("bconv", [P, XT], f32, kind="ExternalInput")
    wxT = nc.dram_tensor("wxT", [DH, NX], f16, kind="ExternalInput")
    wdtT = nc.dram_tensor("wdtT", [DT_RANK, DH], f16, kind="ExternalInput")
    bdt = nc.dram_tensor("bdt", [P, ZT], f32, kind="ExternalInput")
    At = nc.dram_tensor("At", [P, ZT * N_STATE], f32, kind="ExternalInput")
    dskd = nc.dram_tensor("dskd", [P, ZT, P], f16, kind="ExternalInput")
    woutT = nc.dram_tensor("woutT", [P, KT, ZT, P], f16, kind="ExternalInput")
    out = nc.dram_tensor("out", [DM, L], f32, kind="ExternalOutput")

    ident_dr = nc.inline_tensor(np.eye(P, dtype=np.float16), "ident")

    with tile.TileContext(nc) as tc:
        with tc.tile_pool(name="res", bufs=1) as res, \
             tc.tile_pool(name="wpool", bufs=4) as wpool, \
             tc.tile_pool(name="bcp", bufs=3) as bcp, \
             tc.tile_pool(name="wk", bufs=2) as wk, \
             tc.tile_pool(name="scw", bufs=3) as scw, \
             tc.tile_pool(name="dram", bufs=2, space="DRAM") as dram, \
             tc.tile_pool(name="ps", bufs=2, space="PSUM") as ps:

            cc_in = dram.tile([NX, L], f16)
            cc_out = dram.tile([NX, L], f16)

            # ---- resident tiles ----
            xT_sb = res.tile([P, KT, L], f16)       # x^T, k-tile major
            xi = res.tile([P, XT, 3 + L], f16)      # pre-conv xi (3 halo cols)
            xc = res.tile([P, XT, L], f16)          # silu(conv(xi)) = u
            sz = res.tile([P, ZT, L], f16)          # silu(z)
            dt = res.tile([P, ZT, L], f16)          # softplus dt
            y2 = res.tile([P, ZT, L], f16)          # gated scan output
            xdbl = res.tile([P, L], f16)            # x_dbl rows (96 used)
            ident = res.tile([P, P], f16)
            At_sb = res.tile([P, ZT * N_STATE], f32)
            bdt_sb = res.tile([P, ZT], f32)
            wcv_sb = res.tile([P, XT, K_CONV], f32)
            bcv_sb = res.tile([P, XT], f32)
            dskd_sb = res.tile([P, ZT, P], f16)
            wdt_sb = res.tile([DT_RANK, DH], f16)

            nc.sync.dma_start(ident[:], ident_dr[:])
            nc.sync.dma_start(At_sb[:], At[:])
            nc.sync.dma_start(bdt_sb[:], bdt[:])
            nc.sync.dma_start(wcv_sb[:], wcv[:])
            nc.sync.dma_start(dskd_sb[:], dskd[:])
            nc.sync.dma_start(bcv_sb[:], bconv[:])
            nc.sync.dma_start(wdt_sb[:], wdtT[:])
            for q in range(2):
                for k in range(KT):
                    nc.sync.dma_start(
                        xT_sb[:, k, q * FD:(q + 1) * FD],
                        xT[k * P:(k + 1) * P, q * FD:(q + 1) * FD])
            for i in range(XT):
                nc.gpsimd.memset(xi[:, i, 0:3], 0.0)

            # ---- Phase A: in_proj own half (PE) -> ACT evac; conv on DVE
            def emit_inproj(e):
                pacc = ps.tile([P, L], f32, tag="mm")
                wcol = wpool.tile([P, KT, P], f16, tag="wcol")
                for k in range(KT):
                    nc.sync.dma_start(wcol[:, k, :], winT[:, e, k, :])
                for k in range(KT):
                    for f in range(NF):
                        nc.tensor.matmul(
                            pacc[:, f * FD:(f + 1) * FD], wcol[:, k, :],
                            xT_sb[:, k, f * FD:(f + 1) * FD],
                            start=(k == 0), stop=(k == KT - 1))
                nc.scalar.copy(xi[:, e, 3:3 + L], pacc[:])

            def emit_conv(i):
                # tap j=3 (unshifted) writes the full range via 2x
                # tensor_scalar, then j=0..2 accumulate in place
                cacc = wk.tile([P, L], f16, tag="cacc", bufs=6)
                nc.vector.tensor_scalar(
                    cacc[:], xi[:, i, 3:3 + L], wcv_sb[:, i, 3:4],
                    None, MULT)
                for j in range(K_CONV - 1):
                    nc.vector.scalar_tensor_tensor(
                        cacc[:], xi[:, i, j:j + L],
                        wcv_sb[:, i, j:j + 1], cacc[:], MULT, ADD)
                nc.scalar.activation(xc[:, i, :], cacc[:], ACT.Silu,
                                     bias=bcv_sb[:, i:i + 1])

            for e in range(XT):
                emit_inproj(e)
                if e >= 4:
                    emit_conv(e - 4)
            for i in range(XT - 4, XT):
                emit_conv(i)

            # ---- Phase B: partial x_dbl, pair AllReduce, dt ----
            pxd = ps.tile([P, L], f32, tag="mm")
            for i in range(XT):
                wchunk = wpool.tile([P, NX], f16, tag="wx")
                nc.sync.dma_start(wchunk[:], wxT[i * P:(i + 1) * P, :])
                for f in range(NF):
                    nc.tensor.matmul(
                        pxd[:NX, f * FD:(f + 1) * FD], wchunk[:],
                        xc[:, i, f * FD:(f + 1) * FD],
                        start=(i == 0), stop=(i == XT - 1))
            nc.scalar.copy(xdbl[:NX, :], pxd[:NX, :])
            for q in range(4):
                nc.sync.dma_start(
                    cc_in[:, q * (L // 4):(q + 1) * (L // 4)],
                    xdbl[:NX, q * (L // 4):(q + 1) * (L // 4)])
            nc.gpsimd.collective_compute(
                "AllReduce",
                mybir.AluOpType.add,
                replica_groups=[[0, 1], [2, 3], [4, 5], [6, 7]],
                ins=[cc_in.opt()],
                outs=[cc_out.opt()],
            )

            # z projection fills the PE/ACT during the AllReduce wait
            def emit_z(zi):
                pacc = ps.tile([P, L], f32, tag="mm")
                wcol = wpool.tile([P, KT, P], f16, tag="wcol")
                for k in range(KT):
                    nc.sync.dma_start(wcol[:, k, :], winT[:, XT + zi, k, :])
                for k in range(KT):
                    for f in range(NF):
                        nc.tensor.matmul(
                            pacc[:, f * FD:(f + 1) * FD], wcol[:, k, :],
                            xT_sb[:, k, f * FD:(f + 1) * FD],
                            start=(k == 0), stop=(k == KT - 1))
                nc.scalar.activation(sz[:, zi, :], pacc[:], ACT.Silu)

            for zi in range(ZT):
                emit_z(zi)

            for q in range(4):
                nc.sync.dma_start(
                    xdbl[:DT_RANK, q * (L // 4):(q + 1) * (L // 4)],
                    cc_out[:DT_RANK, q * (L // 4):(q + 1) * (L // 4)])

            # ---- Phase C: per d-pair dt, then the selective scan ----
            def emit_dt(d):
                pdt = ps.tile([P, L], f32, tag="mm")
                for f in range(NF):
                    nc.tensor.matmul(
                        pdt[:, f * FD:(f + 1) * FD],
                        wdt_sb[:, d * P:(d + 1) * P],
                        xdbl[:DT_RANK, f * FD:(f + 1) * FD],
                        start=True, stop=True)
                tmp = wk.tile([P, L], f32, tag="f32tmp")
                nc.scalar.activation(tmp[:], pdt[:], ACT.Exp,
                                     bias=bdt_sb[:, d:d + 1])
                nc.scalar.activation(dt[:, d, :], tmp[:], ACT.Ln, bias=1.0)

            BCQ = 4   # broadcast DMA split (latency across engines)
            BF = L // BCQ
            for dp in range(ZT // 2):
                ds = (2 * dp, 2 * dp + 1)
                for d in ds:
                    emit_dt(d)
                yps = {}
                dus = {}
                for d in ds:
                    ypt = ps.tile([P, L], f32, tag="yp")
                    yps[d] = ypt
                    du = wk.tile([P, L], f16, tag="du")
                    nc.vector.tensor_tensor(du[:], dt[:, d, :], xc[:, d, :],
                                            MULT)
                    dus[d] = du
                for n in range(N_STATE):
                    Bn = bcp.tile([P, L], f16, tag="Bn")
                    Cn = bcp.tile([P, L], f16, tag="Cn")
                    for q in range(BCQ):
                        nc.sync.dma_start(
                            Bn[:, q * BF:(q + 1) * BF],
                            cc_out[DT_RANK + n, q * BF:(q + 1) * BF]
                            .partition_broadcast(P))
                        nc.sync.dma_start(
                            Cn[:, q * BF:(q + 1) * BF],
                            cc_out[DT_RANK + N_STATE + n,
                                   q * BF:(q + 1) * BF]
                            .partition_broadcast(P))
                    for d in ds:
                        dA = scw.tile([P, L], f16, tag="dA")
                        nc.scalar.activation(
                            dA[:], dt[:, d, :], ACT.Exp,
                            scale=At_sb[:, d * N_STATE + n:d * N_STATE + n + 1])
                        dBu = scw.tile([P, L], f16, tag="dBu", bufs=2)
                        nc.vector.tensor_tensor(dBu[:], dus[d][:], Bn[:], MULT)
                        H = scw.tile([P, L], f16, tag="H", bufs=2)
                        nc.vector.tensor_tensor_scan(H[:], dA[:], dBu[:], 0.0,
                                                     MULT, ADD)
                        G = scw.tile([P, L], f16, tag="G", bufs=2)
                        nc.vector.tensor_tensor(G[:], H[:], Cn[:], MULT)
                        for f in range(NF):
                            nc.tensor.matmul(
                                yps[d][:, f * FD:(f + 1) * FD], ident[:],
                                G[:, f * FD:(f + 1) * FD],
                                start=(n == 0), stop=False)
                for d in ds:
                    # skip connection: y += diag(Dskip) @ u closes the
                    # accumulation group started by the n-loop
                    for f in range(NF):
                        nc.tensor.matmul(
                            yps[d][:, f * FD:(f + 1) * FD], dskd_sb[:, d, :],
                            xc[:, d, f * FD:(f + 1) * FD],
                            start=False, stop=True)
                    nc.vector.tensor_tensor(y2[:, d, :], yps[d][:],
                                            sz[:, d, :], MULT)

            # ---- Phase D: out_proj partial ----
            for m in range(KT):
                po = ps.tile([P, L], f32, tag="mm")
                wcol = wpool.tile([P, ZT, P], f16, tag="wcol")
                nc.sync.dma_start(wcol[:], woutT[:, m, :, :])
                for k in range(ZT):
                    for f in range(NF):
                        nc.tensor.matmul(
                            po[:, f * FD:(f + 1) * FD], wcol[:, k, :],
                            y2[:, k, f * FD:(f + 1) * FD],
                            start=(k == 0), stop=(k == ZT - 1))
                osb = wk.tile([P, L], f32, tag="f32tmp")
                nc.scalar.copy(osb[:], po[:])
                for q in range(4):
                    nc.sync.dma_start(
                        out[m * P:(m + 1) * P, q * (L // 4):(q + 1) * (L // 4)],
                        osb[:, q * (L // 4):(q + 1) * (L // 4)])

    nc.compile()
    return nc


def _prep_core(inputs, b, rev, half, L=1024, DM=1024, DH=1024):
    """Host-side slicing for one core's in_map (own d_inner half only)."""
    sfx = "r" if rev else "f"
    DI = 2 * DH
    x = np.asarray(inputs["x"])[b].astype(np.float32)     # [L, DM]
    if rev:
        x = x[::-1]
    Win = np.asarray(inputs[f"Win_{sfx}"])
    Wconv = np.asarray(inputs[f"Wconv_{sfx}"])
    bconv = np.asarray(inputs[f"bconv_{sfx}"])
    Wx = np.asarray(inputs[f"Wx_{sfx}"])
    Wdt = np.asarray(inputs[f"Wdt_{sfx}"])
    bdt = np.asarray(inputs[f"bdt_{sfx}"])
    Alog = np.asarray(inputs[f"Alog_{sfx}"])
    Dskip = np.asarray(inputs[f"Dskip_{sfx}"])
    Wout = np.asarray(inputs[f"Wout_{sfx}"])

    own = np.arange(half * DH, (half + 1) * DH)
    XT, ZT = DH // P, DH // P
    KT = DM // P
    ET = XT + ZT

    # e-tiles: own xi rows then own z rows
    winT = np.concatenate(
        [Win[own].T, Win[DI + half * DH:DI + (half + 1) * DH].T], axis=1)
    winT = winT.reshape(KT, P, ET, P).transpose(1, 2, 0, 3)  # [p, e, k, c]
    Wcp = Wconv[own].astype(np.float32)
    wcv = np.ascontiguousarray(
        Wcp.reshape(XT, P, K_CONV).transpose(1, 0, 2))    # [P, XT, K]
    A = -np.exp(Alog[own])                                # [DH, 16]
    pi = np.arange(P)
    dskd = np.zeros((P, ZT, P), np.float16)
    for d in range(ZT):
        dskd[pi, d, pi] = Dskip[own][d * P + pi]
    return {
        "xT": np.ascontiguousarray(x.T).astype(np.float16),
        "winT": np.ascontiguousarray(winT).astype(np.float16),
        "wcv": wcv,
        "bconv": np.ascontiguousarray(
            bconv[own].reshape(XT, P).T).astype(np.float32),
        "wxT": np.ascontiguousarray(Wx[:, own].T).astype(np.float16),
        "wdtT": np.ascontiguousarray(Wdt[own].T).astype(np.float16),
        "bdt": np.ascontiguousarray(
            bdt[own].reshape(ZT, P).T).astype(np.float32),
        "At": np.ascontiguousarray(
            A.reshape(ZT, P, N_STATE).transpose(1, 0, 2).reshape(
                P, ZT * N_STATE)).astype(np.float32),
        "dskd": dskd,
        "woutT": np.ascontiguousarray(Wout[:, own].T.reshape(DH // P, P, DM // P, P).transpose(1, 2, 0, 3)).astype(np.float16),
    }


_NC_CACHE = {}


def kernel(**inputs) -> np.ndarray:
    L, DM = 1024, 1024
    if "nc" not in _NC_CACHE:
        _NC_CACHE["nc"] = build(L=L, DM=DM, DH=1024)
    nc = _NC_CACHE["nc"]

    in_maps = [
        _prep_core(inputs, c // 4, bool((c // 2) % 2), c % 2)
        for c in range(8)
    ]

    import jax
    jax.devices()
    trace = os.environ.get("BIMAMBA_TRACE") == "1"
    if trace:
        from trn_agent_boot.trn_boot import _ntff_profile_via_ctypes
        import antenv.axon_hooks as ah
        if ah.get_axon_ntff_profile_hook() is None:
            ah.set_axon_ntff_profile_hook(
                _ntff_profile_via_ctypes("/opt/axon/libaxon_pjrt.so"))
    res = run_bass_kernel_spmd(nc, in_maps, list(range(8)), trace=trace)
    _NC_CACHE["exec_time_ns"] = res.exec_time_ns

    B = np.asarray(inputs["x"]).shape[0]
    outp = np.zeros((B, L, DM), np.float32)
    for c in range(8):
        b, rev = c // 4, (c // 2) % 2
        part = np.asarray(res.results[c]["out"]).T        # [L, DM]
        if rev:
            part = part[::-1]
        outp[b] += part
    return outp
